# revision 15
# baseline (speedup 1.0000x reference)
"""Local (Gaussian-banded) attention kernel for Trainium2, 8 NeuronCores.

Math: out = rownorm(gauss_band(sigma)) @ (x @ Wg) @ Wout
The Gaussian positional mask with sigma in [0.5, 2.5] decays below fp32
resolution past |i-j| > 32, so attention is a 65-tap banded matmul.

Sharding: core c = (batch b = c//2, seq-half s = c%2). Each core computes
out rows [s*1024, (s+1)*1024) of its batch. s=1 halves are row-reversed on
host so the sequence edge is always at local row 0 -> all 8 cores run the
same program with the same band constants (pure SPMD).

v3: all-bf16 datapath (fp32 PSUM), host-prepacked contiguous DMAs, and
redundant-LDWEIGHTS stripping. The PE weight-load scoreboard only lets
LDWEIGHTS k+2 issue after matmul k fully drains, so back-to-back matmuls
that each reload the stationary pace at ~(mm+ldw)/2 instead of the
stream rate. Consecutive matmuls that reuse one loaded stationary (the
redundant InstLdweights is deleted pre-compile; validated bit-exact on
HW) run at full stream rate:
  stage 1: for each (t,k): one LDW of the xT slice feeds the psA and
           psB matmuls (N=512 each).
  stage 2: 128-col output chunks, window = v[j] (K=128) + v[j+1][:64]
           (K=64): one LDW of v[j] feeds chunk j's A-matmul and chunk
           j-1's C-matmul (both N=128). Interior row-norm is baked into
           the two 128x128 band matrices (same for every chunk); the 32
           sequence-edge rows are fixed by a column rescale on the
           PSUM->SBUF cast output.
  stage 3: out[256,512] = attn chunk @ Wout (8 accumulating N=512
           matmuls; already stream-bound).
Measured rel err vs fp32 reference ~4.5e-3.
"""

import sys

for _p in ("/opt/trn_rl_repo", "/root/.axon_site/_ro/trn_rl_repo"):
    if _p not in sys.path:
        sys.path.append(_p)

import numpy as np
import ml_dtypes

BF16 = ml_dtypes.bfloat16

B, N, D = 4, 2048, 512
H, DH = 8, 128
INNER = H * DH
W = 32                      # band half-width
VROWS = 1088                # 32 zero pad + 1024 own + 32 halo
NT = 9                      # v tiles: 8 x 128 + 1 x 64
REUSE_C = True              # strip LDW on C-pieces (K=64 subset reuse)
STRIP_ON = True             # master switch for LDW stripping
# consts tensor column map (elements, bf16): bandA2 | bandC2 | ec | wout
CA, CC, CE, CW = 0, H * 128, 2 * H * 128, 2 * H * 128 + H * 32
CTOT = CW + H * D

_CACHE = {}


def _build_nc():
    import concourse.mybir as mybir
    from concourse import bacc
    from concourse.tile import TileContext

    f32 = mybir.dt.float32
    bf = mybir.dt.bfloat16

    nc = bacc.Bacc(None, target_bir_lowering=False)
    strip = []

    def mm(out, lhsT, rhs, reuse=False, **kw):
        i = nc.tensor.matmul(out, lhsT, rhs, **kw)
        if reuse and STRIP_ON:
            strip.append(i.ins.name)
        return i

    xT = nc.dram_tensor("xT", [128, 4 * VROWS], bf, kind="ExternalInput")
    wg = nc.dram_tensor("Wg", [128, 4 * INNER], bf, kind="ExternalInput")
    consts = nc.dram_tensor("consts", [128, CTOT], bf, kind="ExternalInput")
    # out rows r = i*256 + half*128 + p  ->  cols i*1024 + half*512 + d
    out = nc.dram_tensor("out", [128, 4 * 1024], bf, kind="ExternalOutput")

    with TileContext(nc) as tc:
        with (
            tc.tile_pool(name="persist", bufs=1) as pp,
            tc.tile_pool(name="outs", bufs=2) as osp,
            tc.tile_pool(name="ps1", bufs=2, space="PSUM") as ps1,
            tc.tile_pool(name="ps2", bufs=2, space="PSUM") as ps2,
            tc.tile_pool(name="ps3", bufs=2, space="PSUM") as ps3,
        ):
            xT_sb = pp.tile([128, 4 * VROWS], bf, tag="xT", name="xT_sb")
            wg_sb = pp.tile([128, 4 * INNER], bf, tag="wg", name="wg_sb")
            cs = pp.tile([128, CTOT], bf, tag="consts", name="cs")
            # SWDGE (gpsimd) emits descriptors ~50x faster than the HWDGE
            # ring, so k-split transfers pipeline: the k=0 slices land
            # first and unblock stage 1 early. consts ride the HWDGE ring
            # in parallel; they are not needed until stage 2.
            for k in range(4):
                nc.gpsimd.dma_start(out=xT_sb[:, k * VROWS:(k + 1) * VROWS],
                                    in_=xT[:, k * VROWS:(k + 1) * VROWS])
                nc.gpsimd.dma_start(out=wg_sb[:, k * INNER:(k + 1) * INNER],
                                    in_=wg[:, k * INNER:(k + 1) * INNER])
            nc.scalar.dma_start(out=cs, in_=consts[:, :])

            v_sb = [pp.tile([128, INNER], bf, tag=f"v{t}", name=f"v{t}")
                    for t in range(NT)]
            # attnT: [dh=128, (h, out col 0..1024)] bf16
            attnT = pp.tile([128, H * 1024], bf, tag="attnT", name="attnT")

            def s1(t):
                rows = 128 if t < 8 else 64
                # one 2-bank tile so the A/B halves allocate atomically and
                # the scheduler cannot split the weight-sharing pairs
                psT = ps1.tile([128, 1024], f32, tag="s1", name=f"psT{t}")
                psA, psB = psT[:, 0:512], psT[:, 512:1024]
                for k in range(4):
                    lh = xT_sb[:, k * VROWS + t * 128: k * VROWS + t * 128 + rows]
                    mm(psA[:rows, :], lh, wg_sb[:, k * INNER:k * INNER + 512],
                       start=(k == 0), stop=(k == 3))
                    mm(psB[:rows, :], lh, wg_sb[:, k * INNER + 512:(k + 1) * INNER],
                       reuse=True, start=(k == 0), stop=(k == 3))
                nc.vector.tensor_copy(v_sb[t][:rows, :], psT[:rows, :])

            def s2(q):
                # sweep bank q: out cols [512q, 512q+512), all heads
                for h in range(H):
                    hs = slice(h * 128, (h + 1) * 128)
                    bk = ps2.tile([128, 512], f32, tag="s2", name=f"s2_{q}_{h}")
                    for j in range(4 * q, 4 * q + 4):
                        c = (j % 4) * 128
                        mm(bk[:, c:c + 128], v_sb[j][:, hs],
                           cs[:, CA + h * 128:CA + (h + 1) * 128],
                           start=(j % 4 == 0), stop=False)
                        if j % 4 > 0:
                            mm(bk[:, c - 128:c], v_sb[j][:64, hs],
                               cs[:64, CC + h * 128:CC + (h + 1) * 128],
                               reuse=REUSE_C, start=False, stop=False)
                    mm(bk[:, 384:512], v_sb[4 * q + 4][:64, hs],
                       cs[:64, CC + h * 128:CC + (h + 1) * 128],
                       start=False, stop=True)
                    at = attnT[:, h * 1024 + q * 512: h * 1024 + q * 512 + 512]
                    nc.vector.tensor_copy(at, bk)
                    if q == 0:
                        # edge rescale: first 32 sequence rows of this head
                        v3 = attnT[:, h * 1024: h * 1024 + 32]
                        nc.vector.tensor_mul(
                            v3, v3, cs[:, CE + h * 32: CE + (h + 1) * 32])

            ots = {}

            def s3(i):
                # chunk pairs share one SBUF tile; DMA out per pair so the
                # transfer has 4KB-per-partition descriptors (2x the rate)
                if i % 2 == 0:
                    ots[i // 2] = osp.tile([128, 2048], bf, tag="outt",
                                           name=f"ot{i // 2}")
                ot = ots[i // 2]
                for half in range(2):
                    ps = ps3.tile([128, 512], f32, tag="s3", name=f"ps3_{i}_{half}")
                    for h in range(H):
                        off = h * 1024 + i * 256 + half * 128
                        mm(ps, attnT[:, off:off + 128],
                           cs[:, CW + h * D: CW + (h + 1) * D],
                           start=(h == 0), stop=(h == 7))
                    nc.vector.tensor_copy(
                        ot[:, (i % 2) * 1024 + half * 512:
                           (i % 2) * 1024 + (half + 1) * 512], ps)
                if i % 2 == 1:
                    nc.gpsimd.dma_start(
                        out=out[:, (i - 1) * 1024:(i + 1) * 1024], in_=ot)

            s1(0); s1(1); s1(2); s1(3); s1(4)
            s2(0)
            s1(5); s1(6)
            s3(0)
            s1(7); s1(8)
            s3(1)
            s2(1)
            s3(2)
            s3(3)

    # Strip redundant InstLdweights: simulate the tensor queue in final
    # block order tracking the loaded stationary; a marked matmul's own
    # LDW is deleted only when the currently-loaded weights already cover
    # it (same tensor/offset/cols, partition-count superset). The Tile
    # scheduler may reorder pairs, so coverage is checked, not assumed.
    import concourse.mybir as mybir
    names = set(strip)

    def sig(ap):
        p = list(ap.ap)
        return (ap.memref, ap.offset, tuple(p[1]), p[0][1], p[0][0])

    def covers(loaded, w):
        return (loaded is not None and loaded[0] == w[0] and loaded[1] == w[1]
                and loaded[2] == w[2] and loaded[4] == w[4]
                and w[3] <= loaded[3])

    removed = 0
    kept = 0
    for blk in nc.m.functions[0].blocks:
        insts = blk.instructions
        loaded = None
        pend = None          # (idx, sig) of LDW awaiting its matmul
        dels = []
        for idx in range(len(insts)):
            inst = insts[idx]
            if isinstance(inst, mybir.InstLdweights):
                si = inst.sync_info
                assert pend is None, "two LDWs with no matmul between"
                pend = (idx, sig(inst.ins[0]),
                        si is None or len(si.on_wait) == 0)
            elif isinstance(inst, mybir.InstMatmult):
                w = sig(inst.ins[1])
                if pend is not None:
                    assert pend[1] == w, (pend[1], w)
                    if inst.name in names and covers(loaded, w) and pend[2]:
                        dels.append(pend[0])
                        removed += 1
                    else:
                        loaded = pend[1]
                        if inst.name in names:
                            kept += 1
                    pend = None
                else:
                    assert covers(loaded, w), (loaded, w)
        for idx in reversed(dels):
            del insts[idx]
    if removed + kept:
        sys.stderr.write(f"ldw strip: removed {removed}, kept {kept}\n")
    nc.compile()
    return nc


def _band_constants(sigma: np.ndarray):
    """Unified band matrices (interior row-norm baked in) + edge rescale."""
    sig = np.asarray(sigma, np.float64).reshape(H)
    d = np.arange(W + 1, dtype=np.float64)
    wts = np.exp(-(d[None, :] ** 2) / (2.0 * sig[:, None] ** 2))  # [H, 33]
    tail = wts[:, 1:].sum(1)
    s_int = wts[:, 0] + 2.0 * tail

    r = np.arange(128)
    c = np.arange(128)
    # A: src = padded row 128j+r (pos 128j+r-32), out col 128j+c
    distA = np.abs(c[None, :] - r[:, None] + 32)          # [128 src, 128 out]
    # C: src = padded row 128(j+1)+r2, r2 in [0,64)
    r2 = np.arange(64)
    distC = np.abs(c[None, :] - 96 - r2[:, None])         # [64, 128]

    def bands(dist, m):
        msk = dist <= W
        wp = np.where(msk[None], wts[:, np.minimum(dist, W).astype(int)], 0.0)
        wp = wp / s_int[:, None, None]                    # [H, m, 128]
        return np.ascontiguousarray(
            wp.transpose(1, 0, 2).reshape(m, H * 128)).astype(BF16)

    bandA2 = bands(distA, 128)
    bandC2 = bands(distC, 64)

    # edge rowsum for out rows 0..31 (left-truncated gaussian)
    re = np.arange(32)
    cum = np.concatenate([np.zeros((H, 1)), np.cumsum(wts[:, 1:], 1)], 1)
    s_edge = wts[:, [0]] + cum[:, np.minimum(re, W)] + tail[:, None]  # [H, 32]
    ecv = (s_int[:, None] / s_edge).astype(np.float32)
    ecb = np.ascontiguousarray(
        np.broadcast_to(ecv.reshape(1, H * 32), (128, H * 32))).astype(BF16)
    return bandA2, bandC2, ecb


def _pack_k(a, cols):
    # [512, cols] -> [128, 4*cols] with partition p = d%128, k = d//128
    return np.ascontiguousarray(
        a.reshape(4, 128, cols).transpose(1, 0, 2).reshape(128, 4 * cols))


def _consts(Wg_unused, Wout, sigma):
    bandA2, bandC2, ecb = _band_constants(sigma)
    cs = np.zeros((128, CTOT), BF16)
    cs[:, CA:CC] = bandA2
    cs[:64, CC:CE] = bandC2
    cs[:, CE:CW] = ecb
    cs[:, CW:] = np.asarray(Wout, BF16).reshape(H, 128, D).transpose(1, 0, 2) \
        .reshape(128, H * D)
    return cs


def _in_maps(x, Wg, Wout, sigma):
    cs = _consts(None, Wout, sigma)
    wg = _pack_k(np.asarray(Wg, BF16), INNER)
    x = np.asarray(x, np.float32)
    maps = []
    for c in range(8):
        b, s = divmod(c, 2)
        z = x[b] if s == 0 else x[b, ::-1]
        xbuf = np.zeros((VROWS, D), np.float32)
        xbuf[32:] = z[:1056]
        xT = _pack_k(np.ascontiguousarray(xbuf.T).astype(BF16), VROWS)
        maps.append({"xT": xT, "Wg": wg, "consts": cs})
    return maps


def _get_nc():
    if "nc" not in _CACHE:
        _CACHE["nc"] = _build_nc()
    return _CACHE["nc"]


def run_spmd(in_maps, **kw):
    from concourse.bass_utils import run_bass_kernel_spmd
    return run_bass_kernel_spmd(_get_nc(), in_maps, core_ids=list(range(8)), **kw)


def _assemble(results):
    full = np.empty((B, N, D), np.float32)
    for c in range(8):
        b, s = divmod(c, 2)
        r = results[c]["out"]          # [128, 4096] bf16
        r = r.astype(np.float32).reshape(128, 4, 2, 512)
        r = r.transpose(1, 2, 0, 3).reshape(1024, 512)
        if s == 0:
            full[b, :1024] = r
        else:
            full[b, 1024:] = r[::-1]
    return full


def kernel(x, Wg, Wout, sigma):
    res = run_spmd(_in_maps(x, Wg, Wout, sigma))
    return _assemble(res.results)
